# revision 22
# baseline (speedup 1.0000x reference)
"""TRN2 Bass kernel for nn_CudaSafeLinear: out = input @ weight.T + bias.

Shapes: input [8192, 4096] f32, weight [4096, 4096] f32, bias [4096] f32.
Sharding: data-parallel over batch rows — core c computes rows [1024c, 1024(c+1)).

Per-core GEMM (out^T orientation):
  outT[n, m] = sum_k wT[k, n] * xT[k, m] + bias[n]
with wT = weight.T ([K, N] in DRAM), xT = input_shard.T ([K, 1024]).
Stationary operand = wT k-tiles [128, 128]; moving operand = resident xT
chunks [128, 512]. Matmuls run in float32r (TF32-class precision, full PE
rate at moving dim >= 256). Accumulation is fp32 in PSUM; bias is added on
the Scalar engine during PSUM->SBUF eviction (psum partitions = out
features, so bias is a per-partition scalar).
"""

import numpy as np

import concourse.mybir as mybir
import concourse.tile as tile
from concourse import bacc
from concourse.bass_utils import run_bass_kernel_spmd

B, K, N = 8192, 4096, 4096
NCORES = 8
BC = B // NCORES          # 1024 batch rows per core
P = 128
KT = K // P               # 32 contraction tiles
MCH = BC // 512           # 2 moving chunks of 512
NSUB = N // P             # 32 stationary (out-feature) tiles
F32R = mybir.dt.float32r
F32 = mybir.dt.float32

_cached = {}


def build():
    nc = bacc.Bacc("TRN2", target_bir_lowering=False, debug=False, num_devices=NCORES)
    xT = nc.dram_tensor("xT", [K, BC], F32R, kind="ExternalInput").ap()
    wT = nc.dram_tensor("wT", [K, N], F32R, kind="ExternalInput").ap()
    bias = nc.dram_tensor("bias", [N, 1], F32, kind="ExternalInput").ap()
    outT = nc.dram_tensor("outT", [N, BC], F32, kind="ExternalOutput").ap()
    # Sink for PE warm-up matmuls (keeps them alive through DCE).
    warm_out = nc.dram_tensor("warm_out", [P, 512], F32, kind="ExternalOutput").ap()

    with tile.TileContext(nc) as tc:
        with (
            tc.tile_pool(name="xres", bufs=1) as x_pool,
            tc.tile_pool(name="bres", bufs=1) as b_pool,
            tc.tile_pool(name="w", bufs=20) as w_pool,
            tc.tile_pool(name="ps", bufs=8, space="PSUM") as ps_pool,
            tc.tile_pool(name="ev", bufs=4) as ev_pool,
        ):
            # Resident input shard: 32 k-tiles of [128, 1024] f32r (16.8 MB).
            # Split across the two low-jitter HW-DGE queues (Sync/Scalar) so
            # the load runs at ~2x single-queue bandwidth; the ramp weights
            # ride the GpSimd SWDGE path instead.
            x_tiles = []
            for k in range(KT):
                xt = x_pool.tile([P, BC], F32R, tag=f"x{k}")
                eng = nc.sync if k % 2 == 0 else nc.scalar
                eng.dma_start(xt[:], xT[k * P:(k + 1) * P, :])
                x_tiles.append(xt)
            # Resident bias: [128, 1] per out-feature tile.
            b_tiles = []
            for i in range(NSUB):
                bt = b_pool.tile([P, 1], F32, tag=f"b{i}")
                nc.gpsimd.dma_start(bt[:], bias[i * P:(i + 1) * P, :])
                b_tiles.append(bt)

            def emit_mms(psums, wt, wcol, k, n_group):
                # psums: [len(n_group)][MCH]; stationary = wt[:, 128*(i+wcol)]
                for i in range(len(n_group)):
                    for j in range(MCH):
                        nc.tensor.matmul(
                            psums[i][j][:],
                            wt[:, 128 * (i + wcol):128 * (i + wcol + 1)],
                            x_tiles[k][:, 512 * j:512 * (j + 1)],
                            start=(k == 0),
                            stop=(k == KT - 1),
                        )

            def emit_evict(n_group, psums):
                for i, n_sub in enumerate(n_group):
                    for j in range(MCH):
                        ot = ev_pool.tile([P, 512], F32, tag="ot", name="ot")
                        # Evict on DVE (otherwise idle) so the Scalar and
                        # Sync queues stay dedicated to the weight stream.
                        nc.vector.tensor_scalar_add(
                            ot[:], psums[i][j][:], b_tiles[n_sub][:]
                        )
                        eng = nc.sync if (n_sub + j) % 2 == 0 else nc.scalar
                        eng.dma_start(
                            outT[n_sub * P:(n_sub + 1) * P, 512 * j:512 * (j + 1)],
                            ot[:],
                        )

            def alloc_psums(ng):
                return [
                    [ps_pool.tile([P, 512], F32, tag="ps", name="ps") for _ in range(MCH)]
                    for _ in range(ng)
                ]

            # ---- PE warm-up: dense junk matmuls from t=0 so the HAM clock
            # gate reaches 8/8 (~3.4us of sustained PE activity) before the
            # first real matmul, and the PE never idles while the first
            # input/weight tiles are in flight (~5us).
            junk = ev_pool.tile([P, 512], F32, tag="junk", name="junk", bufs=1)
            junkw = ev_pool.tile([P, 128], F32, tag="junkw", name="junkw", bufs=1)
            nc.vector.memset(junk[:], 0.0)
            nc.vector.memset(junkw[:], 0.0)
            pwarm = ps_pool.tile([P, 512], F32, tag="ps", name="ps")
            for _ in range(16):
                nc.tensor.matmul(
                    pwarm[:],
                    junkw[:].bitcast(F32R),
                    junk[:].bitcast(F32R),
                    start=True,
                    stop=True,
                )
            wsb = ev_pool.tile([P, 512], F32, tag="ot", name="ot")
            nc.vector.tensor_copy(wsb[:], pwarm[:])
            nc.sync.dma_start(warm_out[:], wsb[:])

            # ---- Ramp: n_subs {0,1,2} together, k-major (6 PSUM banks +
            # warm-up bank). 6 real MMs per k-step (~1.36us) matches the x
            # arrival rate (~1.4us/k over the two HW queues), so the PE
            # stays dense and the HAM clock gate holds 8/8 throughout the
            # input load. Ramp weights ride GpSimd's SWDGE path so the HW
            # queues are dedicated to x. A filler every 4th k-step pads
            # residual arrival jitter.
            ramp_group = [0, 1, 2]
            psums_r = alloc_psums(len(ramp_group))
            for k in range(KT):
                wt = w_pool.tile([P, 384], F32R, tag="w", name="w")
                nc.gpsimd.dma_start(wt[:], wT[k * P:(k + 1) * P, 0:384])
                emit_mms(psums_r, wt, 0, k, ramp_group)
                if k % 4 == 3:
                    nc.tensor.matmul(
                        pwarm[:],
                        junkw[:].bitcast(F32R),
                        junk[:].bitcast(F32R),
                        start=True,
                        stop=True,
                    )
            emit_evict(ramp_group, psums_r)

            # ---- n_sub 3 singleton (completes the first 512-col block).
            psums3 = alloc_psums(1)
            for k in range(KT):
                wt = w_pool.tile([P, 128], F32R, tag="w", name="w")
                weng = nc.sync if k % 2 == 0 else nc.scalar
                weng.dma_start(wt[:], wT[k * P:(k + 1) * P, 384:512])
                emit_mms(psums3, wt, 0, k, [3])
            emit_evict([3], psums3)

            # ---- Steady state: one pair of n_subs at a time; weight
            # stream split across both HW-DGE queues (67 MB must sustain
            # ~153 GB/s; one queue peaks at ~188 GB/s and micro-stalls the
            # PE).
            for pair in range(2, NSUB // 2):
                psums = alloc_psums(2)
                n_group = [2 * pair, 2 * pair + 1]
                for k in range(KT):
                    wt = w_pool.tile([P, 256], F32R, tag="w", name="w")
                    weng = nc.sync if k % 2 == 0 else nc.scalar
                    weng.dma_start(
                        wt[:], wT[k * P:(k + 1) * P, 256 * pair:256 * (pair + 1)]
                    )
                    emit_mms(psums, wt, 0, k, n_group)
                emit_evict(n_group, psums)
    nc.compile()
    return nc


def make_in_maps(input, weight, bias):
    x = np.asarray(input, dtype=np.float32)
    w = np.asarray(weight, dtype=np.float32)
    b = np.asarray(bias, dtype=np.float32)
    wT = np.ascontiguousarray(w.T)
    bcol = np.ascontiguousarray(b.reshape(N, 1))
    in_maps = []
    for c in range(NCORES):
        xTc = np.ascontiguousarray(x[c * BC:(c + 1) * BC, :].T)
        in_maps.append({"xT": xTc, "wT": wT, "bias": bcol})
    return in_maps


def gather(results):
    out = np.empty((B, N), dtype=np.float32)
    for c in range(NCORES):
        out[c * BC:(c + 1) * BC, :] = results[c]["outT"].T
    return out


def kernel(input, weight, bias):
    if "nc" not in _cached:
        _cached["nc"] = build()
    nc = _cached["nc"]
    in_maps = make_in_maps(input, weight, bias)
    res = run_bass_kernel_spmd(nc, in_maps, core_ids=list(range(NCORES)))
    return gather(res.results)


# revision 23
# speedup vs baseline: 1.0353x; 1.0353x over previous
"""TRN2 Bass kernel for nn_CudaSafeLinear: out = input @ weight.T + bias.

Shapes: input [8192, 4096] f32, weight [4096, 4096] f32, bias [4096] f32.
Sharding: data-parallel over batch rows — core c computes rows [1024c, 1024(c+1)).

Per-core GEMM (out^T orientation):
  outT[n, m] = sum_k wT[k, n] * xT[k, m] + bias[n]
with wT = weight.T ([K, N] in DRAM), xT = input_shard.T ([K, 1024]).
Stationary operand = wT k-tiles [128, 128]; moving operand = resident xT
chunks [128, 512]. Matmuls run in float32r (TF32-class precision, full PE
rate at moving dim >= 256). Accumulation is fp32 in PSUM; bias is added on
the Scalar engine during PSUM->SBUF eviction (psum partitions = out
features, so bias is a per-partition scalar).
"""

import numpy as np

import concourse.mybir as mybir
import concourse.tile as tile
from concourse import bacc
from concourse.bass_utils import run_bass_kernel_spmd

B, K, N = 8192, 4096, 4096
NCORES = 8
BC = B // NCORES          # 1024 batch rows per core
P = 128
KT = K // P               # 32 contraction tiles
MCH = BC // 512           # 2 moving chunks of 512
NSUB = N // P             # 32 stationary (out-feature) tiles
F32R = mybir.dt.float32r
F32 = mybir.dt.float32

_cached = {}


def build():
    nc = bacc.Bacc("TRN2", target_bir_lowering=False, debug=False, num_devices=NCORES)
    xT = nc.dram_tensor("xT", [K, BC], F32R, kind="ExternalInput").ap()
    wT = nc.dram_tensor("wT", [K, N], F32R, kind="ExternalInput").ap()
    bias = nc.dram_tensor("bias", [N, 1], F32, kind="ExternalInput").ap()
    outT = nc.dram_tensor("outT", [N, BC], F32, kind="ExternalOutput").ap()
    # Sink for PE warm-up matmuls (keeps them alive through DCE).
    warm_out = nc.dram_tensor("warm_out", [P, 512], F32, kind="ExternalOutput").ap()

    with tile.TileContext(nc) as tc:
        with (
            tc.tile_pool(name="xres", bufs=1) as x_pool,
            tc.tile_pool(name="bres", bufs=1) as b_pool,
            tc.tile_pool(name="w", bufs=20) as w_pool,
            tc.tile_pool(name="ps", bufs=8, space="PSUM") as ps_pool,
            tc.tile_pool(name="ev", bufs=4) as ev_pool,
        ):
            # Resident input shard: 32 k-tiles of [128, 1024] f32r (16.8 MB).
            # Split across the two low-jitter HW-DGE queues (Sync/Scalar) so
            # the load runs at ~2x single-queue bandwidth; the ramp weights
            # ride the GpSimd SWDGE path instead.
            x_tiles = []
            for k in range(KT):
                xt = x_pool.tile([P, BC], F32R, tag=f"x{k}")
                eng = nc.sync if k % 2 == 0 else nc.scalar
                eng.dma_start(xt[:], xT[k * P:(k + 1) * P, :])
                x_tiles.append(xt)
            # Resident bias: [128, 1] per out-feature tile. On the Scalar
            # queue behind the x loads (arrives ~45us, first use ~50us) —
            # NOT on gpsimd, where the 4096 tiny descriptors would stall
            # the SWDGE ring that carries the ramp weights.
            b_tiles = []
            for i in range(NSUB):
                bt = b_pool.tile([P, 1], F32, tag=f"b{i}")
                nc.scalar.dma_start(bt[:], bias[i * P:(i + 1) * P, :])
                b_tiles.append(bt)

            def emit_mms(psums, wt, wcol, k, n_group):
                # psums: [len(n_group)][MCH]; stationary = wt[:, 128*(i+wcol)]
                for i in range(len(n_group)):
                    for j in range(MCH):
                        nc.tensor.matmul(
                            psums[i][j][:],
                            wt[:, 128 * (i + wcol):128 * (i + wcol + 1)],
                            x_tiles[k][:, 512 * j:512 * (j + 1)],
                            start=(k == 0),
                            stop=(k == KT - 1),
                        )

            def emit_evict(n_group, psums):
                for i, n_sub in enumerate(n_group):
                    for j in range(MCH):
                        ot = ev_pool.tile([P, 512], F32, tag="ot", name="ot")
                        # Evict on DVE (otherwise idle) so the Scalar and
                        # Sync queues stay dedicated to the weight stream.
                        nc.vector.tensor_scalar_add(
                            ot[:], psums[i][j][:], b_tiles[n_sub][:]
                        )
                        eng = nc.sync if (n_sub + j) % 2 == 0 else nc.scalar
                        eng.dma_start(
                            outT[n_sub * P:(n_sub + 1) * P, 512 * j:512 * (j + 1)],
                            ot[:],
                        )

            def alloc_psums(ng):
                return [
                    [ps_pool.tile([P, 512], F32, tag="ps", name="ps") for _ in range(MCH)]
                    for _ in range(ng)
                ]

            # ---- PE warm-up: dense junk matmuls from t=0 so the HAM clock
            # gate reaches 8/8 (~3.4us of sustained PE activity) before the
            # first real matmul, and the PE never idles while the first
            # input/weight tiles are in flight (~5us).
            junk = ev_pool.tile([P, 512], F32, tag="junk", name="junk", bufs=1)
            junkw = ev_pool.tile([P, 128], F32, tag="junkw", name="junkw", bufs=1)
            nc.vector.memset(junk[:], 0.0)
            nc.vector.memset(junkw[:], 0.0)
            pwarm = ps_pool.tile([P, 512], F32, tag="ps", name="ps")
            for _ in range(16):
                nc.tensor.matmul(
                    pwarm[:],
                    junkw[:].bitcast(F32R),
                    junk[:].bitcast(F32R),
                    start=True,
                    stop=True,
                )
            wsb = ev_pool.tile([P, 512], F32, tag="ot", name="ot")
            nc.vector.tensor_copy(wsb[:], pwarm[:])
            nc.sync.dma_start(warm_out[:], wsb[:])

            # ---- Ramp: n_subs {0,1,2} together, k-major (6 PSUM banks +
            # warm-up bank). 6 real MMs per k-step (~1.36us) matches the x
            # arrival rate (~1.4us/k over the two HW queues), so the PE
            # stays dense and the HAM clock gate holds 8/8 throughout the
            # input load. Ramp weights ride GpSimd's SWDGE path so the HW
            # queues are dedicated to x. A filler every 4th k-step pads
            # residual arrival jitter.
            ramp_group = [0, 1, 2]
            psums_r = alloc_psums(len(ramp_group))
            for k in range(KT):
                wt = w_pool.tile([P, 384], F32R, tag="w", name="w")
                nc.gpsimd.dma_start(wt[:], wT[k * P:(k + 1) * P, 0:384])
                emit_mms(psums_r, wt, 0, k, ramp_group)
                if k % 4 == 3:
                    nc.tensor.matmul(
                        pwarm[:],
                        junkw[:].bitcast(F32R),
                        junk[:].bitcast(F32R),
                        start=True,
                        stop=True,
                    )
            emit_evict(ramp_group, psums_r)

            # ---- n_sub 3 singleton (completes the first 512-col block).
            psums3 = alloc_psums(1)
            for k in range(KT):
                wt = w_pool.tile([P, 128], F32R, tag="w", name="w")
                weng = nc.sync if k % 2 == 0 else nc.scalar
                weng.dma_start(wt[:], wT[k * P:(k + 1) * P, 384:512])
                emit_mms(psums3, wt, 0, k, [3])
            emit_evict([3], psums3)

            # ---- Steady state: one pair of n_subs at a time; weight
            # stream split across both HW-DGE queues (67 MB must sustain
            # ~153 GB/s; one queue peaks at ~188 GB/s and micro-stalls the
            # PE).
            for pair in range(2, NSUB // 2):
                psums = alloc_psums(2)
                n_group = [2 * pair, 2 * pair + 1]
                for k in range(KT):
                    wt = w_pool.tile([P, 256], F32R, tag="w", name="w")
                    weng = nc.sync if k % 2 == 0 else nc.scalar
                    weng.dma_start(
                        wt[:], wT[k * P:(k + 1) * P, 256 * pair:256 * (pair + 1)]
                    )
                    emit_mms(psums, wt, 0, k, n_group)
                emit_evict(n_group, psums)
    nc.compile()
    return nc


def make_in_maps(input, weight, bias):
    x = np.asarray(input, dtype=np.float32)
    w = np.asarray(weight, dtype=np.float32)
    b = np.asarray(bias, dtype=np.float32)
    wT = np.ascontiguousarray(w.T)
    bcol = np.ascontiguousarray(b.reshape(N, 1))
    in_maps = []
    for c in range(NCORES):
        xTc = np.ascontiguousarray(x[c * BC:(c + 1) * BC, :].T)
        in_maps.append({"xT": xTc, "wT": wT, "bias": bcol})
    return in_maps


def gather(results):
    out = np.empty((B, N), dtype=np.float32)
    for c in range(NCORES):
        out[c * BC:(c + 1) * BC, :] = results[c]["outT"].T
    return out


def kernel(input, weight, bias):
    if "nc" not in _cached:
        _cached["nc"] = build()
    nc = _cached["nc"]
    in_maps = make_in_maps(input, weight, bias)
    res = run_bass_kernel_spmd(nc, in_maps, core_ids=list(range(NCORES)))
    return gather(res.results)
